# revision 13
# baseline (speedup 1.0000x reference)
# Fused attention block (LeViT-style) for Trainium2, 8 NeuronCores, data-parallel over batch.
#
# reference computation (B=16, N=784, DIM=512, H=8, KD=64, VD=256):
#   qkv = BN(x @ qkv_w.T); split q,k,v per head
#   attn = softmax(q @ k.T * KD**-0.5 + attention_biases[:, bias_idxs])
#   out  = BN(silu(attn @ v reshaped) @ proj_w.T)
#
# Strategy (v2 — software-pipelined emission):
#  - batch-parallel: 2 batches per core, weights/bias tables replicated, no collectives
#  - BN folded into weights on host; softmax scale folded into q weights
#  - all matmul operands bf16 (PSUM accumulation fp32), softmax pipeline fp32
#  - scores computed transposed (S^T[j,i]); bias table is symmetric so bias adds unchanged
#  - softmax denominator from an extra ones-column in v (col 256 of each head block)
#  - unstabilized softmax (scores empirically bounded ~|10|, exp is safe in fp32)
#  - heads processed in even/odd pairs at PE row bases 0/64: concurrent row groups
#  - v-bias and proj-bias applied post-normalization via scalar_tensor_tensor with
#    host-broadcast bias rows (no K=1 seed matmuls)
#  - engine split: ACT = exp + bulk silu only; DVE = PSUM evicts/normalize; Pool = bias mult
#  - scores(slot s) interleaved with AV(slot s-1) per j-chunk so PE never waits on exp
#  - PSUM tags: big(2x2banks) scores/passA, av(2) AV/passB, psf(1), pst(1) = 8 banks

import numpy as np
import ml_dtypes

B, N, DIM = 16, 784, 512
H, KD, VD = 8, 64, 256
RES = 28
EPS = 1e-5
SCALE = KD ** -0.5
NCORES = 8
BL = B // NCORES          # batches per core
VDA = VD + 1              # v head block with ones column
OVW = H * VDA             # 2056
NJP = 896                 # padded j extent (7 * 128)

CHUNKS = [(i * 128, min(128, N - i * 128)) for i in range((N + 127) // 128)]
ITILES = [(0, 512), (512, N - 512)]

_CACHE = {}


def _build_nc():
    from contextlib import ExitStack
    import concourse.bacc as bacc
    import concourse.tile as tile
    from concourse import mybir
    import concourse.bass as bass

    bf = mybir.dt.bfloat16
    f32 = mybir.dt.float32
    AF = mybir.ActivationFunctionType
    MULT = mybir.AluOpType.mult
    ADD = mybir.AluOpType.add

    nc = bacc.Bacc("TRN2", target_bir_lowering=False, debug=False)

    xT = nc.dram_tensor("xT", [BL, DIM, N], bf, kind="ExternalInput").ap()
    wqk = nc.dram_tensor("wqk", [128, 4, 1024], bf, kind="ExternalInput").ap()
    wv = nc.dram_tensor("wv", [128, 4, H * VD], bf, kind="ExternalInput").ap()
    wp = nc.dram_tensor("wp", [128, 16, DIM], bf, kind="ExternalInput").ap()
    bqk = nc.dram_tensor("bqk", [128, 8], f32, kind="ExternalInput").ap()
    bvb = nc.dram_tensor("bvb", [128, H * VD], bf, kind="ExternalInput").ap()
    bpb = nc.dram_tensor("bpb", [128, DIM], bf, kind="ExternalInput").ap()
    biast = nc.dram_tensor("biast", [H, NJP, N], bf, kind="ExternalInput").ap()
    ident = nc.dram_tensor("ident", [128, 128], bf, kind="ExternalInput").ap()
    out = nc.dram_tensor("out", [BL, N, DIM], f32, kind="ExternalOutput").ap()

    with ExitStack() as ctx:
        tc = ctx.enter_context(tile.TileContext(nc))
        consts = ctx.enter_context(tc.tile_pool(name="consts", bufs=1))
        xpool = ctx.enter_context(tc.tile_pool(name="xpool", bufs=1))
        qkpool = ctx.enter_context(tc.tile_pool(name="qkpool", bufs=2))
        vpool = ctx.enter_context(tc.tile_pool(name="vpool", bufs=1))
        silupool = ctx.enter_context(tc.tile_pool(name="silupool", bufs=1))
        biaspool = ctx.enter_context(tc.tile_pool(name="biaspool", bufs=2))
        ppool = ctx.enter_context(tc.tile_pool(name="ppool", bufs=2))
        fopool = ctx.enter_context(tc.tile_pool(name="fopool", bufs=2))
        tpool = ctx.enter_context(tc.tile_pool(name="tpool", bufs=2))
        smalls = ctx.enter_context(tc.tile_pool(name="smalls", bufs=4))
        pspool = ctx.enter_context(
            tc.tile_pool(name="pspool", bufs=2, space="PSUM"))

        # ---- constants (DMA order matters: wqk+xT first so passA starts early) ----
        wqk_sb = consts.tile([128, 4, 1024], bf)
        # split so passA's q-half can start before the k-half lands
        nc.sync.dma_start(out=wqk_sb[:, :, 0:512], in_=wqk[:, :, 0:512])
        nc.sync.dma_start(out=wqk_sb[:, :, 512:1024], in_=wqk[:, :, 512:1024])

        xT_tiles = {}

        def load_xT(b):
            xT_sb = xpool.tile([128, 4, N], bf, tag="x", name=f"xT{b}")
            xin = bass.AP(
                tensor=xT.tensor,
                offset=xT.offset + b * DIM * N,
                ap=[[N, 128], [128 * N, 4], [1, N]],
            )
            nc.gpsimd.dma_start(out=xT_sb, in_=xin)
            xT_tiles[b] = xT_sb

        load_xT(0)

        bias_tiles = {}

        def load_bias(b, hp):
            ts = []
            for k in range(2):
                h = 2 * hp + k
                bt = biaspool.tile([128, 7, N], bf, tag="bias", name=f"bias{b}_{hp}_{k}")
                bin_full = bass.AP(
                    tensor=biast.tensor,
                    offset=biast.offset + h * NJP * N,
                    ap=[[N, 128], [128 * N, 6], [1, N]],
                )
                nc.sync.dma_start(out=bt[:, 0:6, :], in_=bin_full)
                bin_tail = bass.AP(
                    tensor=biast.tensor,
                    offset=biast.offset + (h * NJP + 768) * N,
                    ap=[[N, 16], [1, N]],
                )
                nc.sync.dma_start(out=bt[:16, 6, :], in_=bin_tail)
                ts.append(bt)
            bias_tiles[(b, hp)] = ts

        bqk_sb = consts.tile([128, 8], f32)
        nc.sync.dma_start(out=bqk_sb, in_=bqk)
        wv_sb = consts.tile([128, 4, H * VD], bf)
        nc.sync.dma_start(out=wv_sb, in_=wv)

        load_bias(0, 0)

        bvb_sb = consts.tile([128, H * VD], bf)
        nc.sync.dma_start(out=bvb_sb, in_=bvb)
        ident_sb = consts.tile([128, 128], bf)
        nc.sync.dma_start(out=ident_sb, in_=ident)
        wp_sb = consts.tile([128, 16, DIM], bf)
        nc.sync.dma_start(out=wp_sb, in_=wp)
        bpb_sb = consts.tile([128, DIM], bf)
        nc.sync.dma_start(out=bpb_sb, in_=bpb)

        qk_tiles = {}
        p_tiles = {}

        v_sb = vpool.tile([128, 7, OVW], bf)
        v_resh = v_sb.rearrange("p t (h d) -> p t h d", d=VDA)
        nc.vector.memset(v_resh[:, :, :, VD:VDA], 1.0)

        silu_sb = silupool.tile([128, 7, H * VD], bf)

        # ---- emission helpers ----
        def passA_group(b, oc):
            if oc == 0:
                qk_tiles[b] = qkpool.tile([128, 8, N], bf, tag="qk", name=f"qk{b}")
            qk = qk_tiles[b]
            x = xT_tiles[b]
            ps = pspool.tile([128, N], f32, tag="big", name=f"psA{b}_{oc}")
            for (i0, isz) in ITILES:
                for cc in range(4):
                    nc.tensor.matmul(
                        ps[:, i0:i0 + isz],
                        lhsT=wqk_sb[:, cc, oc * 128:(oc + 1) * 128],
                        rhs=x[:, cc, i0:i0 + isz],
                        start=(cc == 0),
                        stop=(cc == 3),
                    )
            # Identity is present in every ACT table set: no table switch cost
            nc.scalar.activation(
                out=qk[:, oc, :], in_=ps, func=AF.Identity,
                bias=bqk_sb[:, oc:oc + 1],
            )

        def passB_group(b, tc_i, ovt, evict="dve"):
            t0, tsz = CHUNKS[tc_i]
            x = xT_tiles[b]
            ps = pspool.tile([128, 512], f32, tag="av", name=f"psB{b}_{tc_i}_{ovt}")
            for cc in range(4):
                nc.tensor.matmul(
                    ps[:tsz, :],
                    lhsT=x[:, cc, t0:t0 + tsz],
                    rhs=wv_sb[:, cc, ovt * 512:(ovt + 1) * 512],
                    start=(cc == 0),
                    stop=(cc == 3),
                )
            dst = v_resh[:tsz, tc_i, 2 * ovt:2 * ovt + 2, :VD]
            src = ps[:tsz, :].rearrange("p (h d) -> p h d", d=VD)
            if evict == "dve":
                nc.vector.tensor_copy(out=dst, in_=src)
            else:
                nc.scalar.copy(out=dst, in_=src)

        def scores_step(b, hp, jc):
            j0, jsz = CHUNKS[jc]
            qk = qk_tiles[b]
            if jc == 0:
                p_tiles[(b, hp)] = ppool.tile(
                    [128, 7, 2, N], bf, tag="p", name=f"p{b}_{hp}")
            pt = p_tiles[(b, hp)]
            bts = bias_tiles[(b, hp)]
            pse = pspool.tile([128, N], f32, tag="big", name=f"pse{b}_{hp}_{jc}")
            pso = pspool.tile([128, N], f32, tag="big", name=f"pso{b}_{hp}_{jc}")
            for (i0, isz) in ITILES:
                nc.tensor.matmul(
                    pse[:jsz, i0:i0 + isz],
                    lhsT=qk[0:64, 4 + hp, j0:j0 + jsz],
                    rhs=qk[0:64, hp, i0:i0 + isz],
                    start=True, stop=True,
                )
                nc.tensor.matmul(
                    pso[:jsz, i0:i0 + isz],
                    lhsT=qk[64:128, 4 + hp, j0:j0 + jsz],
                    rhs=qk[64:128, hp, i0:i0 + isz],
                    start=True, stop=True,
                )
            for k, ps in ((0, pse), (1, pso)):
                # exp(S)*exp(bias) == exp(S+bias); biast holds exp(bias)
                nc.scalar.activation(
                    out=pt[:jsz, jc, k, :], in_=ps[:jsz, :], func=AF.Exp,
                )
                nc.vector.tensor_tensor(
                    out=pt[:jsz, jc, k, :], in0=pt[:jsz, jc, k, :],
                    in1=bts[k][:jsz, jc, :], op=MULT,
                )

        def av_step(b, hp, ic):
            i0, isz = CHUNKS[ic]
            pt = p_tiles[(b, hp)]
            for k in range(2):
                h = 2 * hp + k
                ps = pspool.tile([128, 512], f32, tag="av", name=f"psAV{b}_{hp}_{ic}_{k}")
                for jc, (j0, jsz) in enumerate(CHUNKS):
                    nc.tensor.matmul(
                        ps[:isz, :VDA],
                        lhsT=pt[:jsz, jc, k, i0:i0 + isz],
                        rhs=v_sb[:jsz, jc, h * VDA:(h + 1) * VDA],
                        start=(jc == 0),
                        stop=(jc == 6),
                    )
                rs = smalls.tile([128, 1], f32, tag="rs", name=f"rs{b}_{hp}_{ic}_{k}")
                nc.vector.reciprocal(out=rs[:isz], in_=ps[:isz, VD:VDA])
                # normalized pre-silu values + folded v-bias (softmax rows sum to 1)
                nc.vector.scalar_tensor_tensor(
                    out=silu_sb[:isz, ic, h * VD:(h + 1) * VD],
                    in0=ps[:isz, :VD], scalar=rs[:isz, 0:1],
                    in1=bvb_sb[:isz, h * VD:(h + 1) * VD],
                    op0=MULT, op1=ADD,
                )

        def silu_one(b, tc_i, gate=None):
            t0, tsz = CHUNKS[tc_i]
            kw = {}
            if gate is not None:
                # gate holds 1.0; data dep keeps Silu out of the Exp stream
                # (avoids ACT table thrash from scheduler reordering)
                kw["scale"] = gate[:tsz, 0:1]
            nc.scalar.activation(
                out=silu_sb[:tsz, tc_i, :], in_=silu_sb[:tsz, tc_i, :],
                func=AF.Silu, **kw,
            )

        def proj_unit(b, tc_i, st_eng="dve"):
            t0, tsz = CHUNKS[tc_i]
            psf = pspool.tile([128, 512], f32, tag="psf", bufs=1, name=f"psf{b}_{tc_i}")
            for vp in range(8):
                pst = pspool.tile([128, 2, 128], bf, tag="pst", bufs=1, name=f"pst{b}_{tc_i}_{vp}")
                for k in range(2):
                    vc = vp * 2 + k
                    nc.tensor.transpose(
                        pst[:, k, :tsz],
                        silu_sb[:tsz, tc_i, vc * 128:(vc + 1) * 128],
                        ident_sb[:tsz, :tsz],
                    )
                st = tpool.tile([128, 2, 128], bf, tag="st", name=f"st{b}_{tc_i}_{vp}")
                if st_eng == "dve":
                    nc.vector.tensor_copy(out=st[:, :, :tsz], in_=pst[:, :, :tsz])
                else:
                    nc.scalar.copy(out=st[:, :, :tsz], in_=pst[:, :, :tsz])
                for k in range(2):
                    vc = vp * 2 + k
                    nc.tensor.matmul(
                        psf[:tsz, :],
                        lhsT=st[:, k, :tsz],
                        rhs=wp_sb[:, vc, :],
                        start=(vc == 0),
                        stop=(vc == 15),
                    )
            fo = fopool.tile([128, DIM], f32, tag="fo", name=f"fo{b}_{tc_i}")
            nc.vector.scalar_tensor_tensor(
                out=fo[:tsz], in0=psf[:tsz], scalar=1.0,
                in1=bpb_sb[:tsz], op0=MULT, op1=ADD,
            )
            nc.gpsimd.dma_start(out=out[b, t0:t0 + tsz, :], in_=fo[:tsz])

        # ---- orchestration ----
        for oc in range(8):
            passA_group(0, oc)

        # slot 0: scores(0,hp0) + passB(0)
        pb0 = [(t, o) for o in range(4) for t in range(7)]
        load_bias(0, 1)
        for jc in range(7):
            scores_step(0, 0, jc)
            for g in pb0[4 * jc:4 * jc + 4]:
                passB_group(0, *g)

        # slots 1..3: scores(0,hp) + AV(0,hp-1); spread passA(1)
        passA1 = list(range(8))
        load_xT(1)
        for hp in (1, 2, 3):
            load_bias(0, hp + 1) if hp < 3 else load_bias(1, 0)
            for jc in range(7):
                scores_step(0, hp, jc)
                av_step(0, hp - 1, jc)
                if passA1 and (hp > 1 or jc >= 2):
                    passA_group(1, passA1.pop(0))

        # slot 4: scores(1,hp0) + AV(0,hp3) + passB(1) (evicts on ACT: DVE busy)
        load_bias(1, 1)
        pb1_main = [(t, o) for o in range(3) for t in range(7)]
        for jc in range(7):
            scores_step(1, 0, jc)
            av_step(0, 3, jc)
            for g in pb1_main[3 * jc:3 * jc + 3]:
                passB_group(1, *g, evict="act")
        for t in range(7):
            passB_group(1, t, 3, evict="act")
        # gate = 1.0 with a data dep on the last exp+mult of slot 4, so the
        # scheduler cannot pull Silu into the Exp stream (ACT table thrash)
        gate = smalls.tile([128, 1], f32, tag="gate", bufs=1, name="gate0")
        nc.vector.tensor_scalar(
            out=gate, in0=p_tiles[(1, 0)][:, 6, 1, 0:1],
            scalar1=0.0, scalar2=1.0, op0=MULT, op1=ADD,
        )
        for t in range(7):
            silu_one(0, t, gate=gate)

        # slot 5: scores(1,hp1) + proj(0) + AV(1,hp0)
        load_bias(1, 2)
        for jc in range(7):
            scores_step(1, 1, jc)
            proj_unit(0, jc, st_eng="dve")
            av_step(1, 0, jc)

        # slot 6: scores(1,hp2) + AV(1,hp1)
        load_bias(1, 3)
        for jc in range(7):
            scores_step(1, 2, jc)
            av_step(1, 1, jc)

        # slot 7: scores(1,hp3) + AV(1,hp2)
        for jc in range(7):
            scores_step(1, 3, jc)
            av_step(1, 2, jc)

        # slot 8 / tail: AV(1,hp3) + silu + proj, proj lagging one chunk so
        # AV matmuls fill the PE while DVE drains the transpose copies
        for ic in range(7):
            av_step(1, 3, ic)
            silu_one(1, ic)
            if ic > 0:
                proj_unit(1, ic - 1, st_eng="dve")
        proj_unit(1, 6, st_eng="dve")

    nc.finalize()
    return nc


def _prep(inputs):
    bf16 = ml_dtypes.bfloat16
    f32 = np.float32
    inputs = {k: np.asarray(v) for k, v in inputs.items()}

    s_qkv = (inputs["qkv_gamma"] / np.sqrt(inputs["qkv_var"] + EPS)).astype(f32)
    b_qkv = (inputs["qkv_beta"] - inputs["qkv_mean"] * s_qkv).astype(f32)
    w_fold = (inputs["qkv_w"] * s_qkv[:, None]).astype(f32)

    rows = np.arange((2 * KD + VD) * H).reshape(H, 2 * KD + VD)
    q_rows = rows[:, :KD].ravel()
    k_rows = rows[:, KD:2 * KD].ravel()
    v_rows = rows[:, 2 * KD:].ravel()

    wq = w_fold[q_rows] * SCALE
    bq = b_qkv[q_rows] * SCALE
    wk = w_fold[k_rows]
    bk = b_qkv[k_rows]
    wvm = w_fold[v_rows]
    bvm = b_qkv[v_rows]

    # wqk: [c, o] with o = [q(512), k(512)] -> [128, cc, 1024]
    wqkT = np.concatenate([wq, wk], axis=0).T.astype(bf16)          # [512, 1024]
    wqk_t = np.ascontiguousarray(wqkT.reshape(4, 128, 1024).transpose(1, 0, 2))
    bqk_t = np.concatenate([bq, bk]).reshape(8, 128).T.astype(f32)  # [128, 8]
    bqk_t = np.ascontiguousarray(bqk_t)

    wv_t = np.ascontiguousarray(
        wvm.T.astype(bf16).reshape(4, 128, H * VD).transpose(1, 0, 2)
    )
    # v-bias broadcast row (applied post-normalization)
    bvb_t = np.ascontiguousarray(
        np.broadcast_to(bvm.astype(bf16)[None, :], (128, H * VD)))

    s_p = (inputs["proj_gamma"] / np.sqrt(inputs["proj_var"] + EPS)).astype(f32)
    b_p = (inputs["proj_beta"] - inputs["proj_mean"] * s_p).astype(f32)
    wp_fold = (inputs["proj_w"] * s_p[:, None]).astype(f32)          # [512, 2048]
    wp_t = np.ascontiguousarray(
        wp_fold.T.astype(bf16).reshape(16, 128, DIM).transpose(1, 0, 2)
    )
    bpb_t = np.ascontiguousarray(
        np.broadcast_to(b_p.astype(bf16)[None, :], (128, DIM)))

    bias_full = inputs["attention_biases"][:, inputs["bias_idxs"]].astype(f32)  # [H, N, N]
    biast = np.zeros((H, NJP, N), dtype=bf16)
    biast[:, :N, :] = np.exp(bias_full).astype(bf16)   # multiplicative form

    xT = inputs["x"].transpose(0, 2, 1).astype(bf16)                 # [B, 512, 784]

    shared = {
        "wqk": wqk_t, "wv": wv_t, "wp": wp_t, "bqk": bqk_t,
        "bvb": bvb_t, "bpb": bpb_t, "biast": biast,
        "ident": np.eye(128, dtype=np.float32).astype(bf16),
    }
    in_maps = []
    for c in range(NCORES):
        m = dict(shared)
        m["xT"] = np.ascontiguousarray(xT[c * BL:(c + 1) * BL])
        in_maps.append(m)
    return in_maps


def kernel(trace=False, **inputs):
    from concourse import bass_utils

    if "nc" not in _CACHE:
        _CACHE["nc"] = _build_nc()
    nc = _CACHE["nc"]

    in_maps = _prep(inputs)
    res = bass_utils.run_bass_kernel_spmd(
        nc, in_maps, core_ids=list(range(NCORES)), trace=trace,
    )
    out = np.concatenate([r["out"] for r in res.results], axis=0)
    if trace:
        return out.astype(np.float32), res
    return out.astype(np.float32)


# revision 16
# speedup vs baseline: 1.1398x; 1.1398x over previous
# Fused attention block (LeViT-style) for Trainium2, 8 NeuronCores, data-parallel over batch.
#
# reference computation (B=16, N=784, DIM=512, H=8, KD=64, VD=256):
#   qkv = BN(x @ qkv_w.T); split q,k,v per head
#   attn = softmax(q @ k.T * KD**-0.5 + attention_biases[:, bias_idxs])
#   out  = BN(silu(attn @ v reshaped) @ proj_w.T)
#
# Strategy (v2 — software-pipelined emission):
#  - batch-parallel: 2 batches per core, weights/bias tables replicated, no collectives
#  - BN folded into weights on host; softmax scale folded into q weights
#  - all matmul operands bf16 (PSUM accumulation fp32), softmax pipeline fp32
#  - scores computed transposed (S^T[j,i]); bias table is symmetric so bias adds unchanged
#  - softmax denominator from an extra ones-column in v (col 256 of each head block)
#  - unstabilized softmax (scores empirically bounded ~|10|, exp is safe in fp32)
#  - heads processed in even/odd pairs at PE row bases 0/64: concurrent row groups
#  - v-bias and proj-bias applied post-normalization via scalar_tensor_tensor with
#    host-broadcast bias rows (no K=1 seed matmuls)
#  - engine split: ACT = exp + bulk silu only; DVE = PSUM evicts/normalize; Pool = bias mult
#  - scores(slot s) interleaved with AV(slot s-1) per j-chunk so PE never waits on exp
#  - PSUM tags: big(2x2banks) scores/passA, av(2) AV/passB, psf(1), pst(1) = 8 banks

import numpy as np
import ml_dtypes

B, N, DIM = 16, 784, 512
H, KD, VD = 8, 64, 256
RES = 28
EPS = 1e-5
SCALE = KD ** -0.5
NCORES = 8
BL = B // NCORES          # batches per core
VDA = VD + 1              # v head block with ones column
OVW = H * VDA             # 2056
NJP = 896                 # padded j extent (7 * 128)

CHUNKS = [(i * 128, min(128, N - i * 128)) for i in range((N + 127) // 128)]
ITILES = [(0, 512), (512, N - 512)]

_CACHE = {}


def _build_nc():
    from contextlib import ExitStack
    import concourse.bacc as bacc
    import concourse.tile as tile
    from concourse import mybir
    import concourse.bass as bass

    bf = mybir.dt.bfloat16
    f32 = mybir.dt.float32
    AF = mybir.ActivationFunctionType
    MULT = mybir.AluOpType.mult
    ADD = mybir.AluOpType.add

    nc = bacc.Bacc("TRN2", target_bir_lowering=False, debug=False)

    xT = nc.dram_tensor("xT", [BL, DIM, N], bf, kind="ExternalInput").ap()
    wqk = nc.dram_tensor("wqk", [128, 4, 1024], bf, kind="ExternalInput").ap()
    wv = nc.dram_tensor("wv", [128, 4, H * VD], bf, kind="ExternalInput").ap()
    wp = nc.dram_tensor("wp", [128, 16, DIM], bf, kind="ExternalInput").ap()
    bqk = nc.dram_tensor("bqk", [128, 8], f32, kind="ExternalInput").ap()
    bvb = nc.dram_tensor("bvb", [128, H * VD], bf, kind="ExternalInput").ap()
    bpb = nc.dram_tensor("bpb", [128, DIM], bf, kind="ExternalInput").ap()
    biast = nc.dram_tensor("biast", [H, NJP, N], bf, kind="ExternalInput").ap()
    ident = nc.dram_tensor("ident", [128, 128], bf, kind="ExternalInput").ap()
    out = nc.dram_tensor("out", [BL, N, DIM], f32, kind="ExternalOutput").ap()

    with ExitStack() as ctx:
        tc = ctx.enter_context(tile.TileContext(nc))
        consts = ctx.enter_context(tc.tile_pool(name="consts", bufs=1))
        xpool = ctx.enter_context(tc.tile_pool(name="xpool", bufs=1))
        qkpool = ctx.enter_context(tc.tile_pool(name="qkpool", bufs=2))
        vpool = ctx.enter_context(tc.tile_pool(name="vpool", bufs=1))
        silupool = ctx.enter_context(tc.tile_pool(name="silupool", bufs=1))
        biaspool = ctx.enter_context(tc.tile_pool(name="biaspool", bufs=2))
        ppool = ctx.enter_context(tc.tile_pool(name="ppool", bufs=2))
        fopool = ctx.enter_context(tc.tile_pool(name="fopool", bufs=2))
        tpool = ctx.enter_context(tc.tile_pool(name="tpool", bufs=2))
        smalls = ctx.enter_context(tc.tile_pool(name="smalls", bufs=4))
        pspool = ctx.enter_context(
            tc.tile_pool(name="pspool", bufs=2, space="PSUM"))

        # ---- constants (DMA order matters: wqk+xT first so passA starts early) ----
        wqk_sb = consts.tile([128, 4, 1024], bf)
        # fine-grained so passA's first accumulation steps start ASAP
        for half in (0, 1):
            for cc in range(4):
                nc.sync.dma_start(
                    out=wqk_sb[:, cc, half * 512:(half + 1) * 512],
                    in_=wqk[:, cc, half * 512:(half + 1) * 512])

        xT_tiles = {}

        def load_xT(b):
            xT_sb = xpool.tile([128, 4, N], bf, tag="x", name=f"xT{b}")
            xin = bass.AP(
                tensor=xT.tensor,
                offset=xT.offset + b * DIM * N,
                ap=[[N, 128], [128 * N, 4], [1, N]],
            )
            nc.gpsimd.dma_start(out=xT_sb, in_=xin)
            xT_tiles[b] = xT_sb

        load_xT(0)

        bias_tiles = {}

        def load_bias(b, hp):
            ts = []
            for k in range(2):
                h = 2 * hp + k
                bt = biaspool.tile([128, 7, N], bf, tag="bias", name=f"bias{b}_{hp}_{k}")
                bin_full = bass.AP(
                    tensor=biast.tensor,
                    offset=biast.offset + h * NJP * N,
                    ap=[[N, 128], [128 * N, 6], [1, N]],
                )
                nc.sync.dma_start(out=bt[:, 0:6, :], in_=bin_full)
                bin_tail = bass.AP(
                    tensor=biast.tensor,
                    offset=biast.offset + (h * NJP + 768) * N,
                    ap=[[N, 16], [1, N]],
                )
                nc.sync.dma_start(out=bt[:16, 6, :], in_=bin_tail)
                ts.append(bt)
            bias_tiles[(b, hp)] = ts

        bqk_sb = consts.tile([128, 8], f32)
        nc.sync.dma_start(out=bqk_sb, in_=bqk)
        wv_sb = consts.tile([128, 4, H * VD], bf)
        nc.sync.dma_start(out=wv_sb, in_=wv)

        load_bias(0, 0)

        bvb_sb = consts.tile([128, H * VD], bf)
        nc.sync.dma_start(out=bvb_sb, in_=bvb)
        ident_sb = consts.tile([128, 128], bf)
        nc.sync.dma_start(out=ident_sb, in_=ident)
        wp_sb = consts.tile([128, 16, DIM], bf)
        nc.sync.dma_start(out=wp_sb, in_=wp)
        bpb_sb = consts.tile([128, DIM], bf)
        nc.sync.dma_start(out=bpb_sb, in_=bpb)

        qk_tiles = {}
        p_tiles = {}

        v_sb = vpool.tile([128, 7, OVW], bf)
        v_resh = v_sb.rearrange("p t (h d) -> p t h d", d=VDA)
        nc.vector.memset(v_resh[:, :, :, VD:VDA], 1.0)

        silu_sb = silupool.tile([128, 7, H * VD], bf)

        # ---- emission helpers ----
        def passA_group(b, oc):
            if oc == 0:
                qk_tiles[b] = qkpool.tile([128, 8, N], bf, tag="qk", name=f"qk{b}")
            qk = qk_tiles[b]
            x = xT_tiles[b]
            ps = pspool.tile([128, N], f32, tag="big", name=f"psA{b}_{oc}")
            for (i0, isz) in ITILES:
                for cc in range(4):
                    nc.tensor.matmul(
                        ps[:, i0:i0 + isz],
                        lhsT=wqk_sb[:, cc, oc * 128:(oc + 1) * 128],
                        rhs=x[:, cc, i0:i0 + isz],
                        start=(cc == 0),
                        stop=(cc == 3),
                    )
            # Identity is present in every ACT table set: no table switch cost
            nc.scalar.activation(
                out=qk[:, oc, :], in_=ps, func=AF.Identity,
                bias=bqk_sb[:, oc:oc + 1],
            )

        def passB_group(b, tc_i, ovt, evict="dve"):
            t0, tsz = CHUNKS[tc_i]
            x = xT_tiles[b]
            ps = pspool.tile([128, 512], f32, tag="av", name=f"psB{b}_{tc_i}_{ovt}")
            for cc in range(4):
                nc.tensor.matmul(
                    ps[:tsz, :],
                    lhsT=x[:, cc, t0:t0 + tsz],
                    rhs=wv_sb[:, cc, ovt * 512:(ovt + 1) * 512],
                    start=(cc == 0),
                    stop=(cc == 3),
                )
            dst = v_resh[:tsz, tc_i, 2 * ovt:2 * ovt + 2, :VD]
            src = ps[:tsz, :].rearrange("p (h d) -> p h d", d=VD)
            if evict == "dve":
                nc.vector.tensor_copy(out=dst, in_=src)
            else:
                nc.scalar.copy(out=dst, in_=src)

        def scores_step(b, hp, jc):
            j0, jsz = CHUNKS[jc]
            qk = qk_tiles[b]
            if jc == 0:
                p_tiles[(b, hp)] = ppool.tile(
                    [128, 7, 2, N], bf, tag="p", name=f"p{b}_{hp}")
            pt = p_tiles[(b, hp)]
            bts = bias_tiles[(b, hp)]
            pse = pspool.tile([128, N], f32, tag="big", name=f"pse{b}_{hp}_{jc}")
            pso = pspool.tile([128, N], f32, tag="big", name=f"pso{b}_{hp}_{jc}")
            for (i0, isz) in ITILES:
                nc.tensor.matmul(
                    pse[:jsz, i0:i0 + isz],
                    lhsT=qk[0:64, 4 + hp, j0:j0 + jsz],
                    rhs=qk[0:64, hp, i0:i0 + isz],
                    start=True, stop=True,
                )
                nc.tensor.matmul(
                    pso[:jsz, i0:i0 + isz],
                    lhsT=qk[64:128, 4 + hp, j0:j0 + jsz],
                    rhs=qk[64:128, hp, i0:i0 + isz],
                    start=True, stop=True,
                )
            for k, ps in ((0, pse), (1, pso)):
                # exp(S)*exp(bias) == exp(S+bias); biast holds exp(bias)
                nc.scalar.activation(
                    out=pt[:jsz, jc, k, :], in_=ps[:jsz, :], func=AF.Exp,
                )
                nc.vector.tensor_tensor(
                    out=pt[:jsz, jc, k, :], in0=pt[:jsz, jc, k, :],
                    in1=bts[k][:jsz, jc, :], op=MULT,
                )

        def av_step(b, hp, ic):
            i0, isz = CHUNKS[ic]
            pt = p_tiles[(b, hp)]
            for k in range(2):
                h = 2 * hp + k
                ps = pspool.tile([128, 512], f32, tag="av", name=f"psAV{b}_{hp}_{ic}_{k}")
                for jc, (j0, jsz) in enumerate(CHUNKS):
                    nc.tensor.matmul(
                        ps[:isz, :VDA],
                        lhsT=pt[:jsz, jc, k, i0:i0 + isz],
                        rhs=v_sb[:jsz, jc, h * VDA:(h + 1) * VDA],
                        start=(jc == 0),
                        stop=(jc == 6),
                    )
                rs = smalls.tile([128, 1], f32, tag="rs", name=f"rs{b}_{hp}_{ic}_{k}")
                nc.vector.reciprocal(out=rs[:isz], in_=ps[:isz, VD:VDA])
                # normalized pre-silu values + folded v-bias (softmax rows sum to 1)
                nc.vector.scalar_tensor_tensor(
                    out=silu_sb[:isz, ic, h * VD:(h + 1) * VD],
                    in0=ps[:isz, :VD], scalar=rs[:isz, 0:1],
                    in1=bvb_sb[:isz, h * VD:(h + 1) * VD],
                    op0=MULT, op1=ADD,
                )

        def silu_one(b, tc_i, gate=None):
            t0, tsz = CHUNKS[tc_i]
            kw = {}
            if gate is not None:
                # gate holds 1.0; data dep keeps Silu out of the Exp stream
                # (avoids ACT table thrash from scheduler reordering)
                kw["scale"] = gate[:tsz, 0:1]
            nc.scalar.activation(
                out=silu_sb[:tsz, tc_i, :], in_=silu_sb[:tsz, tc_i, :],
                func=AF.Silu, **kw,
            )

        def proj_unit(b, tc_i, st_eng="dve"):
            t0, tsz = CHUNKS[tc_i]
            psf = pspool.tile([128, 512], f32, tag="psf", bufs=1, name=f"psf{b}_{tc_i}")
            for vp in range(8):
                pst = pspool.tile([128, 2, 128], bf, tag="pst", bufs=1, name=f"pst{b}_{tc_i}_{vp}")
                for k in range(2):
                    vc = vp * 2 + k
                    nc.tensor.transpose(
                        pst[:, k, :tsz],
                        silu_sb[:tsz, tc_i, vc * 128:(vc + 1) * 128],
                        ident_sb[:tsz, :tsz],
                    )
                st = tpool.tile([128, 2, 128], bf, tag="st", name=f"st{b}_{tc_i}_{vp}")
                if st_eng == "dve":
                    nc.vector.tensor_copy(out=st[:, :, :tsz], in_=pst[:, :, :tsz])
                else:
                    nc.scalar.copy(out=st[:, :, :tsz], in_=pst[:, :, :tsz])
                for k in range(2):
                    vc = vp * 2 + k
                    nc.tensor.matmul(
                        psf[:tsz, :],
                        lhsT=st[:, k, :tsz],
                        rhs=wp_sb[:, vc, :],
                        start=(vc == 0),
                        stop=(vc == 15),
                    )
            fo = fopool.tile([128, DIM], f32, tag="fo", name=f"fo{b}_{tc_i}")
            nc.vector.scalar_tensor_tensor(
                out=fo[:tsz], in0=psf[:tsz], scalar=1.0,
                in1=bpb_sb[:tsz], op0=MULT, op1=ADD,
            )
            nc.gpsimd.dma_start(out=out[b, t0:t0 + tsz, :], in_=fo[:tsz])

        # ---- orchestration ----
        for oc in range(8):
            passA_group(0, oc)

        # slot 0: scores(0,hp0) + passB(0)
        pb0 = [(t, o) for o in range(4) for t in range(7)]
        load_bias(0, 1)
        for jc in range(7):
            scores_step(0, 0, jc)
            for g in pb0[4 * jc:4 * jc + 4]:
                passB_group(0, *g)

        # slots 1..3: scores(0,hp) + AV(0,hp-1); spread passA(1)
        passA1 = list(range(8))
        load_xT(1)
        for hp in (1, 2, 3):
            load_bias(0, hp + 1) if hp < 3 else load_bias(1, 0)
            for jc in range(7):
                scores_step(0, hp, jc)
                av_step(0, hp - 1, jc)
                if passA1 and (hp > 1 or jc >= 2):
                    passA_group(1, passA1.pop(0))

        # slot 4: scores(1,hp0) + AV(0,hp3) + passB(1) (evicts on ACT: DVE busy)
        load_bias(1, 1)
        pb1_main = [(t, o) for o in range(3) for t in range(7)]
        for jc in range(7):
            scores_step(1, 0, jc)
            av_step(0, 3, jc)
            for g in pb1_main[3 * jc:3 * jc + 3]:
                passB_group(1, *g, evict="act")
        for t in range(7):
            passB_group(1, t, 3, evict="act")
        # gate = 1.0 with a data dep on the last exp+mult of slot 4, so the
        # scheduler cannot pull Silu into the Exp stream (ACT table thrash)
        gate = smalls.tile([128, 1], f32, tag="gate", bufs=1, name="gate0")
        nc.vector.tensor_scalar(
            out=gate, in0=p_tiles[(1, 0)][:, 6, 1, 0:1],
            scalar1=0.0, scalar2=1.0, op0=MULT, op1=ADD,
        )
        for t in range(7):
            silu_one(0, t, gate=gate)

        # slot 5: proj(0) + scores(1,hp1) + AV(1,hp0); proj first so its
        # transpose-evict copies win DVE priority over mult/norm
        load_bias(1, 2)
        for jc in range(7):
            proj_unit(0, jc, st_eng="dve")
            scores_step(1, 1, jc)
            av_step(1, 0, jc)

        # slot 6: scores(1,hp2) + AV(1,hp1)
        load_bias(1, 3)
        for jc in range(7):
            scores_step(1, 2, jc)
            av_step(1, 1, jc)

        # slot 7: scores(1,hp3) + AV(1,hp2)
        for jc in range(7):
            scores_step(1, 3, jc)
            av_step(1, 2, jc)

        # slot 8 / tail: AV(1,hp3) + silu + proj, proj lagging one chunk and
        # emitted first per iteration so its st copies win DVE priority
        for ic in range(7):
            if ic > 0:
                proj_unit(1, ic - 1, st_eng="dve")
            av_step(1, 3, ic)
            silu_one(1, ic)
        proj_unit(1, 6, st_eng="dve")

    nc.finalize()
    return nc


def _prep(inputs):
    bf16 = ml_dtypes.bfloat16
    f32 = np.float32
    inputs = {k: np.asarray(v) for k, v in inputs.items()}

    s_qkv = (inputs["qkv_gamma"] / np.sqrt(inputs["qkv_var"] + EPS)).astype(f32)
    b_qkv = (inputs["qkv_beta"] - inputs["qkv_mean"] * s_qkv).astype(f32)
    w_fold = (inputs["qkv_w"] * s_qkv[:, None]).astype(f32)

    rows = np.arange((2 * KD + VD) * H).reshape(H, 2 * KD + VD)
    q_rows = rows[:, :KD].ravel()
    k_rows = rows[:, KD:2 * KD].ravel()
    v_rows = rows[:, 2 * KD:].ravel()

    wq = w_fold[q_rows] * SCALE
    bq = b_qkv[q_rows] * SCALE
    wk = w_fold[k_rows]
    bk = b_qkv[k_rows]
    wvm = w_fold[v_rows]
    bvm = b_qkv[v_rows]

    # wqk: [c, o] with o = [q(512), k(512)] -> [128, cc, 1024]
    wqkT = np.concatenate([wq, wk], axis=0).T.astype(bf16)          # [512, 1024]
    wqk_t = np.ascontiguousarray(wqkT.reshape(4, 128, 1024).transpose(1, 0, 2))
    bqk_t = np.concatenate([bq, bk]).reshape(8, 128).T.astype(f32)  # [128, 8]
    bqk_t = np.ascontiguousarray(bqk_t)

    wv_t = np.ascontiguousarray(
        wvm.T.astype(bf16).reshape(4, 128, H * VD).transpose(1, 0, 2)
    )
    # v-bias broadcast row (applied post-normalization)
    bvb_t = np.ascontiguousarray(
        np.broadcast_to(bvm.astype(bf16)[None, :], (128, H * VD)))

    s_p = (inputs["proj_gamma"] / np.sqrt(inputs["proj_var"] + EPS)).astype(f32)
    b_p = (inputs["proj_beta"] - inputs["proj_mean"] * s_p).astype(f32)
    wp_fold = (inputs["proj_w"] * s_p[:, None]).astype(f32)          # [512, 2048]
    wp_t = np.ascontiguousarray(
        wp_fold.T.astype(bf16).reshape(16, 128, DIM).transpose(1, 0, 2)
    )
    bpb_t = np.ascontiguousarray(
        np.broadcast_to(b_p.astype(bf16)[None, :], (128, DIM)))

    bias_full = inputs["attention_biases"][:, inputs["bias_idxs"]].astype(f32)  # [H, N, N]
    biast = np.zeros((H, NJP, N), dtype=bf16)
    biast[:, :N, :] = np.exp(bias_full).astype(bf16)   # multiplicative form

    xT = inputs["x"].transpose(0, 2, 1).astype(bf16)                 # [B, 512, 784]

    shared = {
        "wqk": wqk_t, "wv": wv_t, "wp": wp_t, "bqk": bqk_t,
        "bvb": bvb_t, "bpb": bpb_t, "biast": biast,
        "ident": np.eye(128, dtype=np.float32).astype(bf16),
    }
    in_maps = []
    for c in range(NCORES):
        m = dict(shared)
        m["xT"] = np.ascontiguousarray(xT[c * BL:(c + 1) * BL])
        in_maps.append(m)
    return in_maps


def kernel(trace=False, **inputs):
    from concourse import bass_utils

    if "nc" not in _CACHE:
        _CACHE["nc"] = _build_nc()
    nc = _CACHE["nc"]

    in_maps = _prep(inputs)
    res = bass_utils.run_bass_kernel_spmd(
        nc, in_maps, core_ids=list(range(NCORES)), trace=trace,
    )
    out = np.concatenate([r["out"] for r in res.results], axis=0)
    if trace:
        return out.astype(np.float32), res
    return out.astype(np.float32)


# revision 17
# speedup vs baseline: 1.2316x; 1.0805x over previous
# Fused attention block (LeViT-style) for Trainium2, 8 NeuronCores, data-parallel over batch.
#
# reference computation (B=16, N=784, DIM=512, H=8, KD=64, VD=256):
#   qkv = BN(x @ qkv_w.T); split q,k,v per head
#   attn = softmax(q @ k.T * KD**-0.5 + attention_biases[:, bias_idxs])
#   out  = BN(silu(attn @ v reshaped) @ proj_w.T)
#
# Strategy (v2 — software-pipelined emission):
#  - batch-parallel: 2 batches per core, weights/bias tables replicated, no collectives
#  - BN folded into weights on host; softmax scale folded into q weights
#  - all matmul operands bf16 (PSUM accumulation fp32), softmax pipeline fp32
#  - scores computed transposed (S^T[j,i]); bias table is symmetric so bias adds unchanged
#  - softmax denominator from an extra ones-column in v (col 256 of each head block)
#  - unstabilized softmax (scores empirically bounded ~|10|, exp is safe in fp32)
#  - heads processed in even/odd pairs at PE row bases 0/64: concurrent row groups
#  - v-bias and proj-bias applied post-normalization via scalar_tensor_tensor with
#    host-broadcast bias rows (no K=1 seed matmuls)
#  - engine split: ACT = exp + bulk silu only; DVE = PSUM evicts/normalize; Pool = bias mult
#  - scores(slot s) interleaved with AV(slot s-1) per j-chunk so PE never waits on exp
#  - PSUM tags: big(2x2banks) scores/passA, av(2) AV/passB, psf(1), pst(1) = 8 banks

import numpy as np
import ml_dtypes

B, N, DIM = 16, 784, 512
H, KD, VD = 8, 64, 256
RES = 28
EPS = 1e-5
SCALE = KD ** -0.5
NCORES = 8
BL = B // NCORES          # batches per core
VDA = VD + 1              # v head block with ones column
OVW = H * VDA             # 2056
NJP = 896                 # padded j extent (7 * 128)

CHUNKS = [(i * 128, min(128, N - i * 128)) for i in range((N + 127) // 128)]
ITILES = [(0, 512), (512, N - 512)]

_CACHE = {}


def _build_nc():
    from contextlib import ExitStack
    import concourse.bacc as bacc
    import concourse.tile as tile
    from concourse import mybir
    import concourse.bass as bass

    bf = mybir.dt.bfloat16
    f32 = mybir.dt.float32
    AF = mybir.ActivationFunctionType
    MULT = mybir.AluOpType.mult
    ADD = mybir.AluOpType.add

    nc = bacc.Bacc("TRN2", target_bir_lowering=False, debug=False)

    xT = nc.dram_tensor("xT", [BL, DIM, N], bf, kind="ExternalInput").ap()
    wqk = nc.dram_tensor("wqk", [128, 4, 1024], bf, kind="ExternalInput").ap()
    wv = nc.dram_tensor("wv", [128, 4, H * VD], bf, kind="ExternalInput").ap()
    wp = nc.dram_tensor("wp", [128, 16, DIM], bf, kind="ExternalInput").ap()
    bqk = nc.dram_tensor("bqk", [128, 8], f32, kind="ExternalInput").ap()
    bvb = nc.dram_tensor("bvb", [128, H * VD], bf, kind="ExternalInput").ap()
    bpb = nc.dram_tensor("bpb", [128, DIM], bf, kind="ExternalInput").ap()
    biast = nc.dram_tensor("biast", [H, NJP, N], bf, kind="ExternalInput").ap()
    ident = nc.dram_tensor("ident", [128, 128], bf, kind="ExternalInput").ap()
    out = nc.dram_tensor("out", [BL, N, DIM], f32, kind="ExternalOutput").ap()

    with ExitStack() as ctx:
        tc = ctx.enter_context(tile.TileContext(nc))
        consts = ctx.enter_context(tc.tile_pool(name="consts", bufs=1))
        xpool = ctx.enter_context(tc.tile_pool(name="xpool", bufs=1))
        qkpool = ctx.enter_context(tc.tile_pool(name="qkpool", bufs=2))
        vpool = ctx.enter_context(tc.tile_pool(name="vpool", bufs=1))
        silupool = ctx.enter_context(tc.tile_pool(name="silupool", bufs=1))
        biaspool = ctx.enter_context(tc.tile_pool(name="biaspool", bufs=2))
        ppool = ctx.enter_context(tc.tile_pool(name="ppool", bufs=2))
        fopool = ctx.enter_context(tc.tile_pool(name="fopool", bufs=2))
        tpool = ctx.enter_context(tc.tile_pool(name="tpool", bufs=2))
        smalls = ctx.enter_context(tc.tile_pool(name="smalls", bufs=4))
        pspool = ctx.enter_context(
            tc.tile_pool(name="pspool", bufs=2, space="PSUM"))

        # ---- constants (DMA order matters: wqk+xT first so passA starts early) ----
        wqk_sb = consts.tile([128, 4, 1024], bf)
        # fine-grained so passA's first accumulation steps start ASAP
        for half in (0, 1):
            for cc in range(4):
                nc.sync.dma_start(
                    out=wqk_sb[:, cc, half * 512:(half + 1) * 512],
                    in_=wqk[:, cc, half * 512:(half + 1) * 512])

        xT_tiles = {}

        def load_xT(b):
            xT_sb = xpool.tile([128, 4, N], bf, tag="x", name=f"xT{b}")
            xin = bass.AP(
                tensor=xT.tensor,
                offset=xT.offset + b * DIM * N,
                ap=[[N, 128], [128 * N, 4], [1, N]],
            )
            nc.gpsimd.dma_start(out=xT_sb, in_=xin)
            xT_tiles[b] = xT_sb

        load_xT(0)

        bias_tiles = {}

        def load_bias(b, hp):
            ts = []
            for k in range(2):
                h = 2 * hp + k
                bt = biaspool.tile([128, 7, N], bf, tag="bias", name=f"bias{b}_{hp}_{k}")
                bin_full = bass.AP(
                    tensor=biast.tensor,
                    offset=biast.offset + h * NJP * N,
                    ap=[[N, 128], [128 * N, 6], [1, N]],
                )
                nc.sync.dma_start(out=bt[:, 0:6, :], in_=bin_full)
                bin_tail = bass.AP(
                    tensor=biast.tensor,
                    offset=biast.offset + (h * NJP + 768) * N,
                    ap=[[N, 16], [1, N]],
                )
                nc.sync.dma_start(out=bt[:16, 6, :], in_=bin_tail)
                ts.append(bt)
            bias_tiles[(b, hp)] = ts

        bqk_sb = consts.tile([128, 8], f32)
        nc.sync.dma_start(out=bqk_sb, in_=bqk)
        wv_sb = consts.tile([128, 4, H * VD], bf)
        nc.sync.dma_start(out=wv_sb, in_=wv)

        load_bias(0, 0)

        bvb_sb = consts.tile([128, H * VD], bf)
        nc.sync.dma_start(out=bvb_sb, in_=bvb)
        ident_sb = consts.tile([128, 128], bf)
        nc.sync.dma_start(out=ident_sb, in_=ident)
        wp_sb = consts.tile([128, 16, DIM], bf)
        nc.sync.dma_start(out=wp_sb, in_=wp)
        bpb_sb = consts.tile([128, DIM], bf)
        nc.sync.dma_start(out=bpb_sb, in_=bpb)

        qk_tiles = {}
        p_tiles = {}

        v_sb = vpool.tile([128, 7, OVW], bf)
        v_resh = v_sb.rearrange("p t (h d) -> p t h d", d=VDA)
        nc.vector.memset(v_resh[:, :, :, VD:VDA], 1.0)

        silu_sb = silupool.tile([128, 7, H * VD], bf)

        # ---- emission helpers ----
        def passA_group(b, oc):
            if oc == 0:
                qk_tiles[b] = qkpool.tile([128, 8, N], bf, tag="qk", name=f"qk{b}")
            qk = qk_tiles[b]
            x = xT_tiles[b]
            ps = pspool.tile([128, N], f32, tag="big", name=f"psA{b}_{oc}")
            for (i0, isz) in ITILES:
                for cc in range(4):
                    nc.tensor.matmul(
                        ps[:, i0:i0 + isz],
                        lhsT=wqk_sb[:, cc, oc * 128:(oc + 1) * 128],
                        rhs=x[:, cc, i0:i0 + isz],
                        start=(cc == 0),
                        stop=(cc == 3),
                    )
            # Identity is present in every ACT table set: no table switch cost
            nc.scalar.activation(
                out=qk[:, oc, :], in_=ps, func=AF.Identity,
                bias=bqk_sb[:, oc:oc + 1],
            )

        def passB_group(b, tc_i, ovt, evict="dve"):
            t0, tsz = CHUNKS[tc_i]
            x = xT_tiles[b]
            ps = pspool.tile([128, 512], f32, tag="av", name=f"psB{b}_{tc_i}_{ovt}")
            for cc in range(4):
                nc.tensor.matmul(
                    ps[:tsz, :],
                    lhsT=x[:, cc, t0:t0 + tsz],
                    rhs=wv_sb[:, cc, ovt * 512:(ovt + 1) * 512],
                    start=(cc == 0),
                    stop=(cc == 3),
                )
            dst = v_resh[:tsz, tc_i, 2 * ovt:2 * ovt + 2, :VD]
            src = ps[:tsz, :].rearrange("p (h d) -> p h d", d=VD)
            if evict == "dve":
                nc.vector.tensor_copy(out=dst, in_=src)
            else:
                nc.scalar.copy(out=dst, in_=src)

        def scores_step(b, hp, jc):
            j0, jsz = CHUNKS[jc]
            qk = qk_tiles[b]
            if jc == 0:
                p_tiles[(b, hp)] = ppool.tile(
                    [128, 7, 2, N], bf, tag="p", name=f"p{b}_{hp}")
            pt = p_tiles[(b, hp)]
            bts = bias_tiles[(b, hp)]
            pse = pspool.tile([128, N], f32, tag="big", name=f"pse{b}_{hp}_{jc}")
            pso = pspool.tile([128, N], f32, tag="big", name=f"pso{b}_{hp}_{jc}")
            for (i0, isz) in ITILES:
                nc.tensor.matmul(
                    pse[:jsz, i0:i0 + isz],
                    lhsT=qk[0:64, 4 + hp, j0:j0 + jsz],
                    rhs=qk[0:64, hp, i0:i0 + isz],
                    start=True, stop=True,
                )
                nc.tensor.matmul(
                    pso[:jsz, i0:i0 + isz],
                    lhsT=qk[64:128, 4 + hp, j0:j0 + jsz],
                    rhs=qk[64:128, hp, i0:i0 + isz],
                    start=True, stop=True,
                )
            for k, ps in ((0, pse), (1, pso)):
                # exp(S)*exp(bias) == exp(S+bias); biast holds exp(bias)
                nc.scalar.activation(
                    out=pt[:jsz, jc, k, :], in_=ps[:jsz, :], func=AF.Exp,
                )
                nc.vector.tensor_tensor(
                    out=pt[:jsz, jc, k, :], in0=pt[:jsz, jc, k, :],
                    in1=bts[k][:jsz, jc, :], op=MULT,
                )

        def av_step(b, hp, ic):
            i0, isz = CHUNKS[ic]
            pt = p_tiles[(b, hp)]
            for k in range(2):
                h = 2 * hp + k
                ps = pspool.tile([128, 512], f32, tag="av", name=f"psAV{b}_{hp}_{ic}_{k}")
                for jc, (j0, jsz) in enumerate(CHUNKS):
                    nc.tensor.matmul(
                        ps[:isz, :VDA],
                        lhsT=pt[:jsz, jc, k, i0:i0 + isz],
                        rhs=v_sb[:jsz, jc, h * VDA:(h + 1) * VDA],
                        start=(jc == 0),
                        stop=(jc == 6),
                    )
                rs = smalls.tile([128, 1], f32, tag="rs", name=f"rs{b}_{hp}_{ic}_{k}")
                nc.vector.reciprocal(out=rs[:isz], in_=ps[:isz, VD:VDA])
                # normalized pre-silu values + folded v-bias (softmax rows sum to 1)
                nc.vector.scalar_tensor_tensor(
                    out=silu_sb[:isz, ic, h * VD:(h + 1) * VD],
                    in0=ps[:isz, :VD], scalar=rs[:isz, 0:1],
                    in1=bvb_sb[:isz, h * VD:(h + 1) * VD],
                    op0=MULT, op1=ADD,
                )

        def silu_one(b, tc_i, gate=None):
            t0, tsz = CHUNKS[tc_i]
            kw = {}
            if gate is not None:
                # gate holds 1.0; data dep keeps Silu out of the Exp stream
                # (avoids ACT table thrash from scheduler reordering)
                kw["scale"] = gate[:tsz, 0:1]
            nc.scalar.activation(
                out=silu_sb[:tsz, tc_i, :], in_=silu_sb[:tsz, tc_i, :],
                func=AF.Silu, **kw,
            )

        def proj_unit(b, tc_i, st_eng="dve"):
            t0, tsz = CHUNKS[tc_i]
            psf = pspool.tile([128, 512], f32, tag="psf", bufs=1, name=f"psf{b}_{tc_i}")
            for vq in range(4):
                pst = pspool.tile([128, 4, 128], bf, tag="pst", bufs=1, name=f"pst{b}_{tc_i}_{vq}")
                for k in range(4):
                    vc = vq * 4 + k
                    nc.tensor.transpose(
                        pst[:, k, :tsz],
                        silu_sb[:tsz, tc_i, vc * 128:(vc + 1) * 128],
                        ident_sb[:tsz, :tsz],
                    )
                st = tpool.tile([128, 4, 128], bf, tag="st", name=f"st{b}_{tc_i}_{vq}")
                if st_eng == "dve":
                    nc.vector.tensor_copy(out=st[:, :, :tsz], in_=pst[:, :, :tsz])
                else:
                    nc.scalar.copy(out=st[:, :, :tsz], in_=pst[:, :, :tsz])
                for k in range(4):
                    vc = vq * 4 + k
                    nc.tensor.matmul(
                        psf[:tsz, :],
                        lhsT=st[:, k, :tsz],
                        rhs=wp_sb[:, vc, :],
                        start=(vc == 0),
                        stop=(vc == 15),
                    )
            fo = fopool.tile([128, DIM], f32, tag="fo", name=f"fo{b}_{tc_i}")
            nc.vector.scalar_tensor_tensor(
                out=fo[:tsz], in0=psf[:tsz], scalar=1.0,
                in1=bpb_sb[:tsz], op0=MULT, op1=ADD,
            )
            nc.gpsimd.dma_start(out=out[b, t0:t0 + tsz, :], in_=fo[:tsz])

        # ---- orchestration ----
        for oc in range(8):
            passA_group(0, oc)

        # slot 0: scores(0,hp0) + passB(0)
        pb0 = [(t, o) for o in range(4) for t in range(7)]
        load_bias(0, 1)
        for jc in range(7):
            scores_step(0, 0, jc)
            for g in pb0[4 * jc:4 * jc + 4]:
                passB_group(0, *g)

        # slots 1..3: scores(0,hp) + AV(0,hp-1); spread passA(1)
        passA1 = list(range(8))
        load_xT(1)
        for hp in (1, 2, 3):
            load_bias(0, hp + 1) if hp < 3 else load_bias(1, 0)
            for jc in range(7):
                scores_step(0, hp, jc)
                av_step(0, hp - 1, jc)
                if passA1 and (hp > 1 or jc >= 2):
                    passA_group(1, passA1.pop(0))

        # slot 4: scores(1,hp0) + AV(0,hp3) + passB(1) (evicts on ACT: DVE busy)
        load_bias(1, 1)
        pb1_main = [(t, o) for o in range(3) for t in range(7)]
        for jc in range(7):
            scores_step(1, 0, jc)
            av_step(0, 3, jc)
            for g in pb1_main[3 * jc:3 * jc + 3]:
                passB_group(1, *g, evict="act")
        for t in range(7):
            passB_group(1, t, 3, evict="act")
        # gate = 1.0 with a data dep on the last exp+mult of slot 4, so the
        # scheduler cannot pull Silu into the Exp stream (ACT table thrash)
        gate = smalls.tile([128, 1], f32, tag="gate", bufs=1, name="gate0")
        nc.vector.tensor_scalar(
            out=gate, in0=p_tiles[(1, 0)][:, 6, 1, 0:1],
            scalar1=0.0, scalar2=1.0, op0=MULT, op1=ADD,
        )
        for t in range(7):
            silu_one(0, t, gate=gate)

        # slot 5: proj(0) + scores(1,hp1) + AV(1,hp0); proj first so its
        # transpose-evict copies win DVE priority over mult/norm
        load_bias(1, 2)
        for jc in range(7):
            proj_unit(0, jc, st_eng="dve")
            scores_step(1, 1, jc)
            av_step(1, 0, jc)

        # slot 6: scores(1,hp2) + AV(1,hp1)
        load_bias(1, 3)
        for jc in range(7):
            scores_step(1, 2, jc)
            av_step(1, 1, jc)

        # slot 7: scores(1,hp3) + AV(1,hp2)
        for jc in range(7):
            scores_step(1, 3, jc)
            av_step(1, 2, jc)

        # slot 8 / tail: AV(1,hp3) + silu + proj, proj lagging one chunk and
        # emitted first per iteration so its st copies win DVE priority
        for ic in range(7):
            if ic > 0:
                proj_unit(1, ic - 1, st_eng="dve")
            av_step(1, 3, ic)
            silu_one(1, ic)
        proj_unit(1, 6, st_eng="dve")

    nc.finalize()
    return nc


def _prep(inputs):
    bf16 = ml_dtypes.bfloat16
    f32 = np.float32
    inputs = {k: np.asarray(v) for k, v in inputs.items()}

    s_qkv = (inputs["qkv_gamma"] / np.sqrt(inputs["qkv_var"] + EPS)).astype(f32)
    b_qkv = (inputs["qkv_beta"] - inputs["qkv_mean"] * s_qkv).astype(f32)
    w_fold = (inputs["qkv_w"] * s_qkv[:, None]).astype(f32)

    rows = np.arange((2 * KD + VD) * H).reshape(H, 2 * KD + VD)
    q_rows = rows[:, :KD].ravel()
    k_rows = rows[:, KD:2 * KD].ravel()
    v_rows = rows[:, 2 * KD:].ravel()

    wq = w_fold[q_rows] * SCALE
    bq = b_qkv[q_rows] * SCALE
    wk = w_fold[k_rows]
    bk = b_qkv[k_rows]
    wvm = w_fold[v_rows]
    bvm = b_qkv[v_rows]

    # wqk: [c, o] with o = [q(512), k(512)] -> [128, cc, 1024]
    wqkT = np.concatenate([wq, wk], axis=0).T.astype(bf16)          # [512, 1024]
    wqk_t = np.ascontiguousarray(wqkT.reshape(4, 128, 1024).transpose(1, 0, 2))
    bqk_t = np.concatenate([bq, bk]).reshape(8, 128).T.astype(f32)  # [128, 8]
    bqk_t = np.ascontiguousarray(bqk_t)

    wv_t = np.ascontiguousarray(
        wvm.T.astype(bf16).reshape(4, 128, H * VD).transpose(1, 0, 2)
    )
    # v-bias broadcast row (applied post-normalization)
    bvb_t = np.ascontiguousarray(
        np.broadcast_to(bvm.astype(bf16)[None, :], (128, H * VD)))

    s_p = (inputs["proj_gamma"] / np.sqrt(inputs["proj_var"] + EPS)).astype(f32)
    b_p = (inputs["proj_beta"] - inputs["proj_mean"] * s_p).astype(f32)
    wp_fold = (inputs["proj_w"] * s_p[:, None]).astype(f32)          # [512, 2048]
    wp_t = np.ascontiguousarray(
        wp_fold.T.astype(bf16).reshape(16, 128, DIM).transpose(1, 0, 2)
    )
    bpb_t = np.ascontiguousarray(
        np.broadcast_to(b_p.astype(bf16)[None, :], (128, DIM)))

    bias_full = inputs["attention_biases"][:, inputs["bias_idxs"]].astype(f32)  # [H, N, N]
    biast = np.zeros((H, NJP, N), dtype=bf16)
    biast[:, :N, :] = np.exp(bias_full).astype(bf16)   # multiplicative form

    xT = inputs["x"].transpose(0, 2, 1).astype(bf16)                 # [B, 512, 784]

    shared = {
        "wqk": wqk_t, "wv": wv_t, "wp": wp_t, "bqk": bqk_t,
        "bvb": bvb_t, "bpb": bpb_t, "biast": biast,
        "ident": np.eye(128, dtype=np.float32).astype(bf16),
    }
    in_maps = []
    for c in range(NCORES):
        m = dict(shared)
        m["xT"] = np.ascontiguousarray(xT[c * BL:(c + 1) * BL])
        in_maps.append(m)
    return in_maps


def kernel(trace=False, **inputs):
    from concourse import bass_utils

    if "nc" not in _CACHE:
        _CACHE["nc"] = _build_nc()
    nc = _CACHE["nc"]

    in_maps = _prep(inputs)
    res = bass_utils.run_bass_kernel_spmd(
        nc, in_maps, core_ids=list(range(NCORES)), trace=trace,
    )
    out = np.concatenate([r["out"] for r in res.results], axis=0)
    if trace:
        return out.astype(np.float32), res
    return out.astype(np.float32)
